# revision 4
# baseline (speedup 1.0000x reference)
"""Bass/Trainium2 kernel for cubic B-spline encoding (nn_BsplineEncoding), v3.

Scatter architecture: instead of densely evaluating all 64 bins per
(point, dim), compute the 4 active cubic B-spline coefficients per
(point, dim) in narrow batched passes, then place them with the GPSIMD
local_scatter instruction (per-partition indexed scatter that also zeroes
the destination tile).

Per core, groups of 1024 points (128 partitions x J=8 points), batched in
supergroups of up to 16 groups for the narrow compute:
  - ACT: xs = 30.5*x + 30.5  (knot-space coordinate, f32)
  - DVE: fl = int16(xs - 0.5)   (floor; off-by-one at exact integers is
    harmless: u=1 of segment m-1 == u=0 of segment m for B-splines)
    u = xs - fl (fp16), coefficients c0..c3 via Horner-ish fp16 chain,
    flat scatter columns = 1 + fl + 195*j + 65*d (+r), converted to int16
  - Pool: local_scatter zeroes the [128, 8*195] fp16 out tile and places
    the 4*24 coefficients per partition
  - ACT: upcast fp16 -> f32 out tile; DVE: copy x into column 0 of each
    (point,dim) block (f32 exact)
  - one contiguous DMA per group writes 8 rows x 195 f32 per partition

Measured on TRN2 (8 cores, 1M points): ~310 us HW exec, rel err 3.0e-4.
Engine balance per 1024-pt group: Pool scatter ~1.87us, ACT upcast ~1.53us,
DMA out ~2.0us/queue; output-write bandwidth is the binding constraint.
"""

import math
import os
import sys
from contextlib import ExitStack

import numpy as np

for _p in ("/opt/trn_rl_repo", "/root/.axon_site/_ro/trn_rl_repo"):
    if os.path.isdir(_p) and _p not in sys.path:
        sys.path.insert(0, _p)

import concourse.bass as bass  # noqa: E402
import concourse.tile as tile  # noqa: E402
from concourse import bacc, mybir  # noqa: E402
from concourse import bass_utils  # noqa: E402

F32 = mybir.dt.float32
F16 = mybir.dt.float16
I16 = mybir.dt.int16
AL = mybir.AluOpType
ACTF = mybir.ActivationFunctionType

N_CORES = 8
D = 3
K = 64
ROW = D * (1 + K)          # 195 f32 per output row
J = 8                      # points per partition per group
GROUP = 128 * J            # 1024 points per group
M = J * D                  # 24 (point,dim) slots per partition per group
NI = 4 * M                 # 96 scatter indices per partition per group
NE = J * ROW               # 1560 out elems per partition per group
SCALE = (K - 3) / 2.0      # 30.5
MAX_SG = 16                # groups per supergroup

OUT_F32 = True             # device-side f32 upcast before DMA


def _host_consts():
    # base column (within the J*ROW out tile) of c0 when fl == 0, per
    # (j, d) slot m = j*D + d: 1 + j*ROW + d*(K+1)
    base = np.zeros((1, M), dtype=np.float32)
    for j in range(J):
        for d in range(D):
            base[0, j * D + d] = 1.0 + j * ROW + d * (K + 1)
    basetile = np.tile(base, (128, MAX_SG))  # [128, MAX_SG*M]
    return basetile


def _split_supergroups(n_groups):
    sizes = []
    left = n_groups
    while left > 0:
        g = min(MAX_SG, left)
        sizes.append(g)
        left -= g
    return sizes


def build_program(npad):
    assert npad % GROUP == 0
    n_groups = npad // GROUP
    nc = bacc.Bacc("TRN2", target_bir_lowering=False, debug=False,
                   num_devices=N_CORES)
    x_d = nc.dram_tensor("x", [npad, D], F32, kind="ExternalInput").ap()
    out_dt = F32 if OUT_F32 else F16
    out_d = nc.dram_tensor("out", [npad, ROW], out_dt,
                           kind="ExternalOutput").ap()
    base_d = nc.dram_tensor("basetile", [128, MAX_SG * M], F32,
                            kind="ExternalInput").ap()

    with tile.TileContext(nc) as tc, ExitStack() as ctx:
        cpool = ctx.enter_context(tc.tile_pool(name="const", bufs=1))
        base_t = cpool.tile([128, MAX_SG * M], F32, tag="base")
        nc.sync.dma_start(base_t[:], base_d[:])
        b_sc = cpool.tile([128, 1], F32, tag="b_sc")
        nc.vector.memset(b_sc[:], SCALE)

        sg_p = ctx.enter_context(tc.tile_pool(name="sg", bufs=3))
        out_p = ctx.enter_context(tc.tile_pool(name="out", bufs=6))
        outf_p = ctx.enter_context(tc.tile_pool(name="outf", bufs=4))

        g0 = 0
        for G in _split_supergroups(n_groups):
            b0 = g0 * GROUP
            W = G * M                     # narrow width this supergroup
            x_sl = x_d[b0:b0 + GROUP * G, :].rearrange(
                "(p k) d -> p (k d)", p=128)
            out_sl = out_d[b0:b0 + GROUP * G, :].rearrange(
                "(p g j) f -> g p (j f)", p=128, j=J)

            xin = sg_p.tile([128, W], F32, tag="xin", name="xin")
            nc.sync.dma_start(xin[:], x_sl)
            xs30 = sg_p.tile([128, W], F32, tag="xs30", name="xs30")
            # xs = 30.5*x + 30.5 in [0, 61)
            nc.scalar.activation(xs30[:], xin[:], ACTF.Identity,
                                 bias=b_sc[:], scale=SCALE)
            fl_i = sg_p.tile([128, W], I16, tag="fl_i", name="fl_i")
            nc.vector.tensor_scalar(fl_i[:], xs30[:], 0.5, None, AL.subtract)
            flf = sg_p.tile([128, W], F32, tag="flf", name="flf")
            nc.vector.tensor_copy(flf[:], fl_i[:])
            u = sg_p.tile([128, W], F16, tag="u", name="u")
            nc.vector.tensor_tensor(u[:], xs30[:], flf[:], AL.subtract)
            flb = sg_p.tile([128, W], F16, tag="flb", name="flb")
            nc.vector.tensor_tensor(flb[:], flf[:], base_t[:, :W], AL.add)

            idxf = sg_p.tile([128, G * NI], F16, tag="idxf", name="idxf")
            iv = idxf[:].rearrange("p (g r m) -> p g r m", g=G, r=4)
            fv = flb[:].rearrange("p (g m) -> p g m", g=G)
            for r in range(4):
                nc.vector.tensor_scalar(iv[:, :, r, :], fv, float(r), None,
                                        AL.add)
            idx16 = sg_p.tile([128, G * NI], I16, tag="idx16", name="idx16")
            nc.vector.tensor_copy(idx16[:], idxf[:])

            u2 = sg_p.tile([128, W], F16, tag="u2", name="u2")
            nc.vector.tensor_tensor(u2[:], u[:], u[:], AL.mult)
            u3 = sg_p.tile([128, W], F16, tag="u3", name="u3")
            nc.vector.tensor_tensor(u3[:], u2[:], u[:], AL.mult)
            v = sg_p.tile([128, W], F16, tag="v", name="v")
            nc.vector.tensor_scalar(v[:], u[:], -1.0, 1.0, AL.mult, AL.add)
            v2 = sg_p.tile([128, W], F16, tag="v2", name="v2")
            nc.vector.tensor_tensor(v2[:], v[:], v[:], AL.mult)
            v3 = sg_p.tile([128, W], F16, tag="v3", name="v3")
            nc.vector.tensor_tensor(v3[:], v2[:], v[:], AL.mult)

            data = sg_p.tile([128, G * NI], F16, tag="data", name="data")
            dv = data[:].rearrange("p (g r m) -> p g r m", g=G, r=4)
            # c0 = (1-u)^3/6, c3 = u^3/6
            nc.vector.tensor_scalar(dv[:, :, 0, :],
                                    v3[:].rearrange("p (g m) -> p g m", g=G),
                                    1.0 / 6.0, None, AL.mult)
            nc.vector.tensor_scalar(dv[:, :, 3, :],
                                    u3[:].rearrange("p (g m) -> p g m", g=G),
                                    1.0 / 6.0, None, AL.mult)
            # c1 = u^3/2 - u^2 + 2/3
            c1t = sg_p.tile([128, W], F16, tag="c1t", name="c1t")
            nc.vector.scalar_tensor_tensor(c1t[:], u3[:], 0.5, u2[:],
                                           AL.mult, AL.subtract)
            nc.vector.tensor_scalar(dv[:, :, 1, :],
                                    c1t[:].rearrange("p (g m) -> p g m", g=G),
                                    2.0 / 3.0, None, AL.add)
            # c2 = 1 - c0 - c1 - c3 (partition of unity)
            s1 = sg_p.tile([128, W], F16, tag="s1", name="s1")
            s1v = s1[:].rearrange("p (g m) -> p g m", g=G)
            nc.vector.tensor_tensor(s1v, dv[:, :, 0, :], dv[:, :, 3, :],
                                    AL.add)
            s2 = sg_p.tile([128, W], F16, tag="s2", name="s2")
            s2v = s2[:].rearrange("p (g m) -> p g m", g=G)
            nc.vector.tensor_tensor(s2v, s1v, dv[:, :, 1, :], AL.add)
            nc.vector.tensor_scalar(dv[:, :, 2, :],
                                    s2[:].rearrange("p (g m) -> p g m", g=G),
                                    -1.0, 1.0, AL.mult, AL.add)

            for g in range(G):
                out_t = out_p.tile([128, NE], F16, tag="out", name="out_t")
                nc.gpsimd.local_scatter(
                    out_t[:], data[:, g * NI:(g + 1) * NI],
                    idx16[:, g * NI:(g + 1) * NI],
                    channels=128, num_elems=NE, num_idxs=NI)
                x_g = xin[:, g * M:(g + 1) * M]
                if OUT_F32:
                    outf = outf_p.tile([128, NE], F32, tag="outf",
                                       name="outf")
                    # fp16->f32 upcast on ACT (own SBUF ports; DVE work
                    # contends with the GPSIMD scatter's shared POOL slot)
                    nc.scalar.copy(outf[:], out_t[:])
                    # x passthrough into column 0 of each (point,dim) block,
                    # f32-exact, after the upcast overwrote the whole tile
                    ovf = outf[:].rearrange("p (j d q) -> p j d q", j=J, d=D)
                    nc.vector.tensor_copy(
                        ovf[:, :, :, 0:1],
                        x_g.rearrange("p (j d) -> p j d", j=J)[:, :, :, None])
                    nc.sync.dma_start(out_sl[g], outf[:])
                else:
                    ov = out_t[:].rearrange("p (j d q) -> p j d q", j=J, d=D)
                    nc.vector.tensor_copy(
                        ov[:, :, :, 0:1],
                        x_g.rearrange("p (j d) -> p j d", j=J)[:, :, :, None])
                    nc.sync.dma_start(out_sl[g], out_t[:])
            g0 += G

    nc.compile()
    return nc


_CACHE = {}


def _get_program(npad):
    if npad not in _CACHE:
        _CACHE[npad] = build_program(npad)
    return _CACHE[npad]


def run_sharded(x, trace=False):
    """x: [N, 3] fp32, N divisible by N_CORES. Returns ([N,195] f32, results)."""
    n = x.shape[0]
    assert n % N_CORES == 0
    nsh = n // N_CORES
    npad = int(math.ceil(nsh / GROUP)) * GROUP
    nc = _get_program(npad)
    basetile = _host_consts()
    in_maps = []
    for i in range(N_CORES):
        sh = np.asarray(x[i * nsh:(i + 1) * nsh], dtype=np.float32)
        if npad != nsh:
            sh = np.concatenate(
                [sh, np.zeros((npad - nsh, D), np.float32)], axis=0)
        in_maps.append({
            "x": np.ascontiguousarray(sh),
            "basetile": basetile,
        })
    res = bass_utils.run_bass_kernel_spmd(
        nc, in_maps, core_ids=list(range(N_CORES)), trace=trace)
    outs = []
    for i in range(N_CORES):
        o = res.results[i]["out"][:nsh]
        if o.dtype != np.float32:
            o = o.astype(np.float32)
        outs.append(o)
    return np.concatenate(outs, axis=0), res


def kernel(x):
    x = np.asarray(x, dtype=np.float32)
    out, _ = run_sharded(x, trace=False)
    return out
